# revision 7
# baseline (speedup 1.0000x reference)
"""KimiMoEGate — f16 hi*hi + fp8 DoubleRowSwInterleave crosses (SWI layout).

Per core (1024 tokens): logits = x_hi.w_hi (56 f16 matmuls -> psA) +
(Xh8.Wl8 + Xl8.Wh8)/2^25 (56 fp8-e4m3 DoubleRow matmuls -> psB, both
cross terms fused into one pass via the pair dimension).  Validated on
the real inputs: 3 mismatched tokens, rel_idx 6.7e-3 (same as full
precision).  PE ~72-96 us/core, DMA 36.7 MB ~102 us -> DMA-bound.

Host splits x and w into (f16 hi, fp8 pair) planes; DMA bytes unchanged
vs the f32 input (2+1+1 bytes per element).
"""
import contextlib
import os
import sys
sys.path.insert(0, '/opt/trn_rl_repo')
import numpy as np
import ml_dtypes
import concourse.bass as bass
from concourse import bacc
import concourse.mybir as mybir
from concourse.bass_utils import run_bass_kernel_spmd
from concourse.tile import TileContext

F32 = mybir.dt.float32
F16 = mybir.dt.float16
F8 = mybir.dt.float8e4
U32 = mybir.dt.uint32
I32 = mybir.dt.int32
AX = mybir.AxisListType
ALU = mybir.AluOpType
ACTF = mybir.ActivationFunctionType
DRMODE = mybir.MatmulPerfMode.DoubleRowSwInterleave

T, H, E = 8192, 7168, 256
NCORES = 8
TPC = T // NCORES            # 1024 tokens per core
KT = H // 128                # 56 contraction tiles
NB = TPC // 128              # 8 blocks of 128 tokens
WCH = 14                     # w k-tiles per DMA chunk
NEG = -1e30
CROSS_SCALE = 1.0 / (1 << 25)

_cache = {}
LAST = None


def _build(repeat=1):
    key = ("nc", repeat)
    if key in _cache:
        return _cache[key]
    nc = bacc.Bacc("TRN2", target_bir_lowering=False, debug=False,
                   num_devices=NCORES)
    xhi = nc.dram_tensor("xhi", [128, NB, KT, 128], F16, kind="ExternalInput")
    x8 = nc.dram_tensor("x8", [128, NB, KT, 128, 2], F8, kind="ExternalInput")
    whi = nc.dram_tensor("whi", [128, KT, E], F16, kind="ExternalInput")
    w8 = nc.dram_tensor("w8", [128, KT, 2, E], F8, kind="ExternalInput")
    bias = nc.dram_tensor("bias", [E], F32, kind="ExternalInput")
    o_idx = nc.dram_tensor("o_idx", [TPC, 8], I32, kind="ExternalOutput")
    o_w = nc.dram_tensor("o_w", [TPC, 8], F32, kind="ExternalOutput")

    with TileContext(nc) as tc:
        with (
            tc.tile_pool(name="wpool", bufs=1) as wpool,
            tc.tile_pool(name="xpool", bufs=3) as xpool,
            tc.tile_pool(name="small", bufs=2) as small,
            tc.tile_pool(name="ps", bufs=2, space="PSUM") as ps,
        ):
            loop = tc.For_i(0, repeat) if repeat > 1 else \
                contextlib.nullcontext()
            with loop:
                whs = wpool.tile([128, KT, E], F16)
                w8s = wpool.tile([128, KT, 2, E], F8)
                bias_rep = wpool.tile([128, E], F32)
                # head: first w_hi chunk, then block-0 x, then the rest
                nc.sync.dma_start(whs[:, 0:WCH], whi[:, 0:WCH])
                xh0 = xpool.tile([128, KT, 128], F16, tag="xh")
                nc.sync.dma_start(xh0[:], xhi[:, 0])
                x80 = xpool.tile([128, KT, 128, 2], F8, tag="x8")
                nc.sync.dma_start(x80[:], x8[:, 0])
                for c in range(WCH, KT, WCH):
                    nc.sync.dma_start(whs[:, c:c + WCH], whi[:, c:c + WCH])
                for c in range(0, KT, WCH):
                    nc.sync.dma_start(w8s[:, c:c + WCH], w8[:, c:c + WCH])
                nc.sync.dma_start(bias_rep[:],
                                  bias[None, :].to_broadcast([128, E]))

                for b in range(NB):
                    if b == 0:
                        xh, x8b = xh0, x80
                    else:
                        xh = xpool.tile([128, KT, 128], F16, tag="xh")
                        nc.sync.dma_start(xh[:], xhi[:, b])
                        x8b = xpool.tile([128, KT, 128, 2], F8, tag="x8")
                        nc.sync.dma_start(x8b[:], x8[:, b])

                    psA = ps.tile([128, E], F32, tag="psA")
                    psB = ps.tile([128, E], F32, tag="psB")
                    for k in range(KT):
                        nc.tensor.matmul(psA[:], xh[:, k], whs[:, k],
                                         start=(k == 0), stop=(k == KT - 1))
                    for k in range(KT):
                        nc.tensor.matmul(psB[:], x8b[:, k], w8s[:, k],
                                         perf_mode=DRMODE,
                                         start=(k == 0), stop=(k == KT - 1))

                    # logits = psA + psB/2^25; sigmoid
                    logA = small.tile([128, E], F32, tag="logA")
                    nc.scalar.activation(logA[:], psA[:], ACTF.Copy)
                    logits = small.tile([128, E], F32, tag="logits")
                    nc.vector.scalar_tensor_tensor(logits[:], psB[:],
                                                   CROSS_SCALE, logA[:],
                                                   op0=ALU.mult, op1=ALU.add)
                    s = small.tile([128, E], F32, tag="s")
                    nc.scalar.activation(s[:], logits[:], ACTF.Sigmoid)
                    sc = small.tile([128, E], F32, tag="sc")
                    nc.vector.tensor_tensor(sc[:], s[:], bias_rep[:], ALU.add)

                    scg = sc[:].rearrange("p (g e) -> p g e", g=8)
                    gm = small.tile([128, 8], F32, tag="gm")
                    nc.vector.tensor_reduce(gm[:], scg, AX.X, ALU.max)
                    scr = small.tile([128, E], F32, tag="scr")
                    nc.vector.match_replace(scr[:], gm[:], sc[:], NEG)
                    gm2 = small.tile([128, 8], F32, tag="gm2")
                    nc.vector.tensor_reduce(
                        gm2[:], scr[:].rearrange("p (g e) -> p g e", g=8),
                        AX.X, ALU.max)
                    gsum = small.tile([128, 8], F32, tag="gsum")
                    nc.vector.tensor_tensor(gsum[:], gm[:], gm2[:], ALU.add)
                    g8 = small.tile([128, 8], F32, tag="g8")
                    nc.vector.max(g8[:], gsum[:])
                    gmask = small.tile([128, 8], F32, tag="gmask")
                    nc.vector.tensor_scalar(gmask[:], gsum[:], g8[:, 3:4],
                                            None, op0=ALU.is_ge)
                    tmp = small.tile([128, E], F32, tag="tmp")
                    nc.vector.tensor_tensor(
                        tmp[:].rearrange("p (g e) -> p g e", g=8), scg,
                        gmask[:, :, None].to_broadcast([128, 8, 32]),
                        ALU.mult)
                    v8 = small.tile([128, 8], F32, tag="v8")
                    nc.vector.max(v8[:], tmp[:])
                    i8 = small.tile([128, 8], U32, tag="i8")
                    nc.vector.max_index(i8[:], v8[:], tmp[:])

                    marked = small.tile([128, E], F32, tag="marked")
                    nc.vector.match_replace(marked[:], v8[:], tmp[:], NEG)
                    possel = small.tile([128, E], F32, tag="possel")
                    nc.vector.tensor_tensor(possel[:], tmp[:], marked[:],
                                            ALU.not_equal)
                    s_sel = small.tile([128, E], F32, tag="s_sel")
                    nc.vector.tensor_tensor(s_sel[:], s[:], possel[:],
                                            ALU.mult)
                    w8v = small.tile([128, 8], F32, tag="w8v")
                    nc.vector.max(w8v[:], s_sel[:])
                    is8 = small.tile([128, 8], U32, tag="is8")
                    nc.vector.max_index(is8[:], w8v[:], s_sel[:])

                    eq = small.tile([128, 8, 8], F32, tag="eq")
                    nc.vector.tensor_tensor(
                        eq[:],
                        is8[:, None, :].to_broadcast([128, 8, 8]),
                        i8[:, :, None].to_broadcast([128, 8, 8]),
                        ALU.is_equal)
                    prod = small.tile([128, 8, 8], F32, tag="prod")
                    nc.vector.tensor_tensor(
                        prod[:], eq[:],
                        w8v[:, None, :].to_broadcast([128, 8, 8]), ALU.mult)
                    w8o = small.tile([128, 8], F32, tag="w8o")
                    nc.vector.tensor_reduce(w8o[:], prod[:], AX.X, ALU.add)
                    ssum = small.tile([128, 1], F32, tag="ssum")
                    nc.vector.tensor_reduce(ssum[:], w8v[:], AX.X, ALU.add)
                    rec = small.tile([128, 1], F32, tag="rec")
                    nc.vector.reciprocal(rec[:], ssum[:])
                    rec25 = small.tile([128, 1], F32, tag="rec25")
                    nc.vector.tensor_scalar(rec25[:], rec[:], 2.5, None,
                                            op0=ALU.mult)
                    wfin = small.tile([128, 8], F32, tag="wfin")
                    nc.vector.tensor_scalar(wfin[:], w8o[:], rec25[:], None,
                                            op0=ALU.mult)
                    nc.sync.dma_start(o_w[b * 128:(b + 1) * 128], wfin[:])
                    nc.sync.dma_start(o_idx[b * 128:(b + 1) * 128],
                                      i8[:].bitcast(I32))
    nc.compile()
    _cache[key] = nc
    return nc


E4NP = ml_dtypes.float8_e4m3


def prep_in_maps(hidden_states, weight, e_score_correction_bias):
    x = np.asarray(hidden_states, dtype=np.float32)
    w = np.asarray(weight, dtype=np.float32)
    b = np.asarray(e_score_correction_bias, dtype=np.float32)

    # w planes: [p, k, e]
    wt = w.T.reshape(KT, 128, E).transpose(1, 0, 2)
    wh32 = wt.astype(np.float16).astype(np.float32)
    whi = np.ascontiguousarray(wt.astype(np.float16))
    wl8 = ((wt - wh32) * (1 << 20)).astype(E4NP)          # pair0
    wh8 = (wh32 * 2048.0).astype(E4NP)                    # pair1
    w8 = np.ascontiguousarray(np.stack([wl8, wh8], axis=2))  # [p,k,2,e]

    in_maps = []
    for c in range(NCORES):
        xs = x[c * TPC:(c + 1) * TPC]                     # [1024, H]
        xt = xs.T.reshape(KT, 128, NB, 128).transpose(1, 2, 0, 3)
        xh32 = xt.astype(np.float16).astype(np.float32)   # [p,b,k,n]
        xhi = np.ascontiguousarray(xt.astype(np.float16))
        xh8 = (xh32 * 32.0).astype(E4NP)                  # pair0
        xl8 = ((xt - xh32) * 16384.0).astype(E4NP)        # pair1
        # SWI layout: pairs interleaved per column, columns reversed
        x8 = np.ascontiguousarray(
            np.stack([xh8, xl8], axis=4)[:, :, :, ::-1, :])  # [p,b,k,n,2]
        in_maps.append({"xhi": xhi, "x8": x8, "whi": whi, "w8": w8,
                        "bias": b})
    return in_maps


def kernel(hidden_states, weight, e_score_correction_bias):
    global LAST
    nc = _build()
    in_maps = prep_in_maps(hidden_states, weight, e_score_correction_bias)
    res = run_bass_kernel_spmd(nc, in_maps, list(range(NCORES)))
    LAST = res
    r = res.results
    idx = np.concatenate([r[c]["o_idx"] for c in range(NCORES)], axis=0)
    wgt = np.concatenate([r[c]["o_w"] for c in range(NCORES)], axis=0)
    return idx.astype(np.int32), wgt.astype(np.float32)
